# revision 14
# baseline (speedup 1.0000x reference)
"""Multi-head attention (no mask) Trainium2 kernel, SPMD over 8 NeuronCores.

Baseline structure + bisect stage A: GROUPS=(3,3,3,3,3,1), ps_big bufs=2.
"""

import os
import sys

import numpy as np

for _p in ("/opt/trn_rl_repo", "/root/.axon_site/_ro/trn_rl_repo"):
    if _p not in sys.path and os.path.isdir(_p):
        sys.path.append(_p)

from contextlib import ExitStack

import concourse.bass as bass
import concourse.tile as tile
from concourse import bacc, masks, mybir
from concourse.bass_utils import run_bass_kernel_spmd

FP32 = mybir.dt.float32
BF16 = mybir.dt.bfloat16

N_CORES = 8
B, S, D = 2, 2048, 1024
COLS = 128            # output columns per core = 2 heads x 64
HD = 64               # head dim
SCALE = 0.125         # 1 / sqrt(HD)
QCH = 512             # q chunk (psum free dim)
KCH = 128             # k chunk (partition dim)
NKC = S // KCH        # 16
NQC = S // QCH        # 4
NJ = QCH // 128       # 4 transpose blocks per q chunk
DT = D // 128         # 8 contraction tiles for projections
GROUPS = (2,) * 8             # kchunks per exp block (psum bank budget)
GSLOT = max(GROUPS)
PS_BUFS = 3

_CACHED_NC = None


def build_nc(reps=1):
    nc = bacc.Bacc("TRN2", target_bir_lowering=False, debug=False,
                   num_devices=N_CORES)

    x = nc.dram_tensor("x", [B, S, D], FP32, kind="ExternalInput").ap()
    w_ap = {}
    b_ap = {}
    for p in ("q", "k", "v"):
        w_ap[p] = nc.dram_tensor(f"w{p}", [D, COLS], FP32,
                                 kind="ExternalInput").ap()
        b_ap[p] = nc.dram_tensor(f"b{p}", [COLS], FP32,
                                 kind="ExternalInput").ap()
    out = nc.dram_tensor("out", [B, S, COLS], FP32, kind="ExternalOutput").ap()

    with tile.TileContext(nc) as tc, ExitStack() as ctx:
        dram_pool = ctx.enter_context(tc.tile_pool(name="dram", bufs=1,
                                                   space="DRAM"))
        const_pool = ctx.enter_context(tc.tile_pool(name="const", bufs=1))
        w_pool = ctx.enter_context(tc.tile_pool(name="w", bufs=1))
        xt_pool = ctx.enter_context(tc.tile_pool(name="xt", bufs=2))
        qkv_pool = ctx.enter_context(tc.tile_pool(name="qkv", bufs=2))
        v65_pool = ctx.enter_context(tc.tile_pool(name="v65", bufs=2))
        att_pool = ctx.enter_context(tc.tile_pool(name="att", bufs=4))
        fin_pool = ctx.enter_context(tc.tile_pool(name="fin", bufs=4))
        yout_pool = ctx.enter_context(tc.tile_pool(name="yout", bufs=2))
        ps_big = ctx.enter_context(tc.tile_pool(name="psbig", bufs=PS_BUFS,
                                                space="PSUM"))
        ps_acc = ctx.enter_context(tc.tile_pool(name="psacc", bufs=2,
                                                space="PSUM"))

        id_f32 = const_pool.tile([128, 128], FP32, tag="idf")
        id_bf16 = const_pool.tile([128, 128], BF16, tag="idb")
        masks.make_identity(nc, id_f32[:])
        masks.make_identity(nc, id_bf16[:])

        # Weights (cast fp32->bf16 during DMA) and biases.
        w_sb = {}
        b_sb = {}
        for p in ("q", "k", "v"):
            wt = w_pool.tile([128, DT, COLS], BF16, tag=f"w{p}")
            for t in range(DT):
                nc.gpsimd.dma_start(out=wt[:, t, :],
                                    in_=w_ap[p][t * 128:(t + 1) * 128, :])
            w_sb[p] = wt
            bt = w_pool.tile([COLS, 1], FP32, tag=f"b{p}")
            nc.sync.dma_start(out=bt[:],
                              in_=b_ap[p].rearrange("(p one) -> p one", one=1))
            b_sb[p] = bt

        # bf16 copies of x in DRAM (enable the 2-byte HWDGE DMA transpose).
        x16 = [dram_pool.tile([S, D], BF16, name=f"x16_{b}")
               for b in range(B)]
        state = {}  # keyed (rep, b, name); entries dropped once consumed

        def emit_cast(r, b):
            for c in range(4):
                nc.gpsimd.dma_start(
                    out=x16[b][c * 512:(c + 1) * 512, :].rearrange(
                        "s (u v) -> (s u) v", u=4),
                    in_=x[b, c * 512:(c + 1) * 512, :].rearrange(
                        "s (u v) -> (s u) v", u=4))

        def emit_xt(r, b, half):
            if (r, b, "xt") not in state:
                state[r, b, "xt"] = xt_pool.tile([128, DT, S], BF16,
                                                 tag="xt", name="xt")
            xt = state[r, b, "xt"]
            x16v = x16[b].rearrange("(hh s) (t p) -> hh s t p", p=128,
                                    hh=2)
            for t in range(DT):
                nc.sync.dma_start(out=xt[:, t, half * 1024:(half + 1) * 1024],
                                  in_=x16v[half, :, t], transpose=True)

        def emit_proj(r, b, p, sc):
            if (r, b, p) not in state:
                state[r, b, p] = qkv_pool.tile([128, S], BF16, tag=f"{p}T",
                                               name=f"{p}T")
            pt = state[r, b, p]
            xt = state[r, b, "xt"]
            ps = ps_big.tile([128, QCH], FP32, tag="big", name="psproj")
            for t in range(DT):
                nc.tensor.matmul(
                    ps[:], lhsT=w_sb[p][:, t, :],
                    rhs=xt[:, t, sc * QCH:(sc + 1) * QCH],
                    start=(t == 0), stop=(t == DT - 1))
            nc.vector.tensor_scalar_add(
                pt[:, sc * QCH:(sc + 1) * QCH], ps[:], b_sb[p][:])

        def emit_v65(r, b, kc0, n):
            if (r, b, "v65") not in state:
                v65 = v65_pool.tile([128, NKC, 130], BF16, tag="v65",
                                    name="v65")
                nc.vector.memset(v65[:, :, 64], 1.0)
                nc.vector.memset(v65[:, :, 129], 1.0)
                state[r, b, "v65"] = v65
            v65 = state[r, b, "v65"]
            for kc in range(kc0, kc0 + n):
                pvt = ps_big.tile([128, 128], BF16, tag="big", name="psvt")
                nc.tensor.transpose(pvt[:],
                                    state[r, b, "v"][:,
                                                     kc * 128:(kc + 1) * 128],
                                    id_bf16[:])
                nc.vector.tensor_copy(v65[:, kc, 0:64], pvt[:, 0:64])
                nc.vector.tensor_copy(v65[:, kc, 65:129], pvt[:, 64:128])

        def emit_attn_qc(r, b, qc, hook):
            qT, kT, v65 = (state[r, b, "q"], state[r, b, "k"],
                           state[r, b, "v65"])
            psy = [ps_acc.tile([65, QCH], FP32, tag="acc", name="psy")
                   for _ in range(2)]
            def emit_group_scores(kc0, g):
                pss = [ps_big.tile([128, GSLOT, QCH], FP32, tag="big",
                                   name="pss") for _ in range(2)]
                for j in range(g):
                    kc = kc0 + j
                    for h in range(2):
                        nc.tensor.matmul(
                            pss[h][:, j, :],
                            lhsT=kT[h * HD:(h + 1) * HD,
                                    kc * 128:(kc + 1) * 128],
                            rhs=qT[h * HD:(h + 1) * HD,
                                   qc * QCH:(qc + 1) * QCH],
                            start=True, stop=True)
                att = []
                for h in range(2):
                    at = att_pool.tile([128, GSLOT, QCH], BF16, tag="att",
                                       name="att")
                    nc.scalar.activation(
                        at[:, 0:g, :], pss[h][:, 0:g, :],
                        mybir.ActivationFunctionType.Exp, scale=SCALE)
                    att.append(at)
                return att

            def emit_group_y(att, kc0, g):
                for j in range(g):
                    kc = kc0 + j
                    for h in range(2):
                        nc.tensor.matmul(
                            psy[h][:],
                            lhsT=v65[:, kc, h * 65:(h + 1) * 65],
                            rhs=att[h][:, j, :],
                            start=(kc == 0), stop=(kc == NKC - 1))

            kc0 = 0
            prev = None
            for g in GROUPS:
                att = emit_group_scores(kc0, g)
                if prev is not None:
                    emit_group_y(*prev)
                    if hook is not None:
                        hook()
                prev = (att, kc0, g)
                kc0 += g
            emit_group_y(*prev)
            if hook is not None:
                hook()

            # Finalize this q chunk: transpose yT -> y, normalize, store.
            yo = yout_pool.tile([128, NJ, COLS], FP32, tag="yo", name="yo")
            for h in range(2):
                ysb = fin_pool.tile([65, QCH], FP32, tag="ysb", name="ysb")
                nc.vector.tensor_copy(ysb[:], psy[h][:])
                for j in range(NJ):
                    pyt = ps_big.tile([128, 65], FP32, tag="big", name="psyt")
                    nc.tensor.transpose(pyt[:], ysb[:, j * 128:(j + 1) * 128],
                                        id_f32[0:65, 0:65])
                    rc = fin_pool.tile([128, 1], FP32, tag="rc", name="rc")
                    nc.vector.reciprocal(rc[:], pyt[:, 64:65])
                    nc.vector.tensor_scalar_mul(
                        yo[:, j, h * HD:(h + 1) * HD], pyt[:, 0:64], rc[:])
            nc.sync.dma_start(
                out=out[b, qc * QCH:(qc + 1) * QCH, :].rearrange(
                    "(j p) c -> p j c", p=128),
                in_=yo[:])

        # Batch 0 prologue, then batch 0 attention with batch 1's
        # prologue interleaved through the scheduler via emission order.
        # Cross-rep pipelined schedule: batch bb's prep work (cast, xT
        # DMA-transposes, k/v projections, v65 build) is interleaved as PE
        # filler into the PREVIOUS attention phase, keeping the tensor
        # engine continuously busy (any PE idle gap drops it to the 1.2GHz
        # p-state for the next ~3us). Phase sequence:
        #   prep(r0,b0) | attn(r0,b0)+prep(r0,b1) | attn(r0,b1)+prep(r1,b0)
        #   | attn(r1,b0)+prep(r1,b1) | ...
        def prep_units(r, b):
            units = [lambda: emit_cast(r, b),
                     lambda: emit_xt(r, b, 0), lambda: emit_xt(r, b, 1)]
            for p in ("k", "v"):
                for sc in range(NQC):
                    units.append(lambda p=p, sc=sc: emit_proj(r, b, p, sc))
            for kc0 in range(0, NKC, 4):
                units.append(lambda kc0=kc0: emit_v65(r, b, kc0, 4))
            return units

        pending = []
        hook_n = [0]

        def hook():
            # Pop ~2 of every 3 hook slots so filler work spreads across
            # the whole attention phase instead of front-loading.
            hook_n[0] += 1
            if pending and hook_n[0] % 3 != 2:
                pending.pop(0)()

        for u in prep_units(0, 0):
            u()
        for _rep in range(reps):
            for bb in range(B):
                nxt = (_rep, 1) if bb == 0 else (_rep + 1, 0)
                if nxt[0] < reps:
                    pending.extend(prep_units(*nxt))
                for qc in range(NQC):
                    emit_proj(_rep, bb, "q", qc)
                    emit_attn_qc(_rep, bb, qc, hook)
                while pending:
                    pending.pop(0)()
                # drop tiles this batch no longer needs
                for k in [k for k in state
                          if k[:2] == (_rep, bb) or k[0] < _rep]:
                    del state[k]

    nc.compile()
    return nc


def get_nc():
    global _CACHED_NC
    if _CACHED_NC is None:
        _CACHED_NC = build_nc()
    return _CACHED_NC


def make_in_maps(x, wq, bq, wk, bk, wv, bv):
    in_maps = []
    for i in range(N_CORES):
        c0 = i * COLS
        in_maps.append({
            "x": np.ascontiguousarray(x, dtype=np.float32),
            "wq": np.ascontiguousarray(wq[:, c0:c0 + COLS], dtype=np.float32),
            "wk": np.ascontiguousarray(wk[:, c0:c0 + COLS], dtype=np.float32),
            "wv": np.ascontiguousarray(wv[:, c0:c0 + COLS], dtype=np.float32),
            "bq": np.ascontiguousarray(bq[c0:c0 + COLS], dtype=np.float32),
            "bk": np.ascontiguousarray(bk[c0:c0 + COLS], dtype=np.float32),
            "bv": np.ascontiguousarray(bv[c0:c0 + COLS], dtype=np.float32),
        })
    return in_maps


def kernel(x, wq, bq, wk, bk, wv, bv):
    nc = get_nc()
    in_maps = make_in_maps(x, wq, bq, wk, bk, wv, bv)
    res = run_bass_kernel_spmd(nc, in_maps, list(range(N_CORES)))
    parts = [res.results[i]["out"] for i in range(N_CORES)]
    out = np.concatenate(parts, axis=2).astype(np.float32)
    kernel.last_results = res
    return out
